# revision 1
# baseline (speedup 1.0000x reference)
"""GCN (6-layer GCNConv) Trainium2 Bass kernel — v6.

Data-parallel over batch (1 mesh per NeuronCore). Per layer
out = A_hat @ (x @ W) + b with A_hat = D^-1/2 (A+I) D^-1/2 shared across
batch and layers.

v6 structure (HW: 1.457 ms vs 4.62 ms f32 baseline; rel err 7.8e-3;
KBASS_BUFS=7 + 3 PSUM scatter banks let the 4 SWDGE queue pairs actually
run concurrently):
  - bf16 datapath (PE 1 cyc/row), f32 PSUM accumulation.
  - One 768-index dma_gather per dst tile (indices int16, wrapped in 16
    partitions and replicated to all eight 16-partition stripes — each
    GPSIMD cpu reads its own stripe). Gathers round-robin over 4 SWDGE
    queues in lock-step with the scheduler's DMASW lane rotation.
  - h tables pre-scaled by dinv^2; self-loop + PSUM drain fused into one
    DVE add (node = pa + hres).
  - One-hot scatter matrices built per tile on the otherwise-idle DVE
    ((slot==iota)*norm, bf16 2x mode); streaming them from DRAM instead
    measured slower (DMA contention stretches the gather critical path).
  - Bias via a reserved gather slot per tile (fixed flat slot (C-1)*128):
    its index points at a bias row appended to each h table and its
    one-hot row is memset to all-ones, so the segment-sum matmul adds the
    layer bias for free.
  - Per-tile PSUM->SBUF stage copy is a single wide activation (ReLU
    folded where the reference has it).
"""
import sys
import time

sys.path.insert(0, "/opt/trn_rl_repo")
import numpy as np
import ml_dtypes
from contextlib import ExitStack

import concourse.bass as bass
import concourse.bacc as bacc
import concourse.mybir as mybir
import concourse.tile as tile
from concourse.bass_utils import run_bass_kernel_spmd
from concourse.masks import make_identity

P = 128
F32 = mybir.dt.float32
BF16 = mybir.dt.bfloat16
I16 = mybir.dt.int16
BF = ml_dtypes.bfloat16


def _pack_graph(src, dst, N):
    """Relabel nodes into degree-balanced 128-node tiles. Every tile
    reserves one gather slot (the 'bias slot'): its index points at the
    bias row (row NP) of the gather table and its one-hot row is all-ones.
    Edge norm is dinv[dst]/dinv[src] (tables store h*dinv^2)."""
    T = (N + P - 1) // P
    NP = T * P
    indeg = np.bincount(dst, minlength=N)          # no-loop in-degree
    C = max(1, int(np.ceil((len(src) + T) / (T * P))))

    order = np.argsort(-indeg, kind="stable")
    while True:
        cap = C * P - 1                             # reserve the bias slot
        load = np.zeros(T, np.int64)
        count = np.zeros(T, np.int64)
        assign = np.empty(N, np.int64)
        ok = True
        for v in order:
            d = int(indeg[v])
            best_t, best_rem = -1, -1
            for t in range(T):
                if count[t] < P:
                    rem = cap - load[t]
                    if rem > best_rem:
                        best_rem, best_t = rem, t
            if best_t < 0 or load[best_t] + d > cap:
                ok = False
                break
            assign[v] = best_t
            load[best_t] += d
            count[best_t] += 1
        if ok:
            break
        C += 1

    perm = np.full(NP, -1, np.int64)
    new_of_old = np.empty(N, np.int64)
    cursor = np.zeros(T, np.int64)
    for v in range(N):
        t = assign[v]
        nid = t * P + cursor[t]
        cursor[t] += 1
        perm[nid] = v
        new_of_old[v] = nid

    # symmetric normalization (degree INCLUDES self-loops, per GCN)
    deg = (indeg + 1).astype(np.float32)
    dinv = (1.0 / np.sqrt(deg, dtype=np.float32)).astype(np.float32)
    norm = (dinv[dst] / dinv[src]).astype(np.float32)

    src_n = new_of_old[src]
    dst_n = new_of_old[dst]
    tile_of_e = dst_n // P
    order_e = np.argsort(tile_of_e, kind="stable")
    src_n, dst_n, norm = src_n[order_e], dst_n[order_e], norm[order_e]
    tile_of_e = tile_of_e[order_e]

    CP = C * P
    gsrc = np.zeros((T, CP), np.int32)
    slot = np.full((T, CP), -1.0, np.float32)       # -1: one-hot row all-zero
    nrm = np.zeros((T, CP), np.float32)
    starts = np.searchsorted(tile_of_e, np.arange(T + 1))
    # bias slot fixed at flat (C-1)*128 (chunk C-1, partition 0 — engines
    # can't address APs starting at partition 127); edges skip that slot
    BSLOT = (C - 1) * P
    pos = np.concatenate([np.arange(BSLOT), np.arange(BSLOT + 1, CP)])
    for t in range(T):
        lo, hi = starts[t], starts[t + 1]
        n_e = hi - lo
        assert n_e <= CP - 1, (t, n_e, CP)
        fs = np.zeros(CP, np.int32)
        fs[pos[:n_e]] = src_n[lo:hi]
        fs[BSLOT] = NP                  # bias slot -> bias row; its all-ones
        gsrc[t] = fs                    # one-hot row is device-built
        slot[t, pos[:n_e]] = (dst_n[lo:hi] - t * P).astype(np.float32)
        nrm[t, pos[:n_e]] = norm[lo:hi]

    # int16 index table: wrapped [i%16, i//16], replicated to all 8 stripes
    SW = CP // 16
    idx16 = np.zeros((P, T * SW), np.int16)
    for t in range(T):
        flat = gsrc[t].astype(np.int16)
        idx16[:, t * SW:(t + 1) * SW] = np.tile(flat.reshape(SW, 16).T, (8, 1))

    dinv_new = np.zeros(NP, np.float32)
    valid = perm >= 0
    dinv_new[valid] = dinv[perm[valid]]
    dinv2 = (dinv_new ** 2).reshape(T, P).T.copy()   # [128, T]

    def dev(a):
        # [T, C, P] -> [P(slot e), T*C] device table
        return np.ascontiguousarray(
            a.reshape(T, C, P).transpose(2, 0, 1).reshape(P, T * C))

    return dict(NP=NP, T=T, C=C, SW=SW, perm=perm,
                dinv2=np.ascontiguousarray(dinv2), idx16=idx16,
                slot=dev(slot).astype(BF), norm=dev(nrm).astype(BF))


def _build_nc(NP, T, C, SW, FM, F5, FO):
    import os
    scratch = int(os.environ.get("KBASS_SCRATCH", "32768"))
    nc = bacc.Bacc("TRN2", dynamic_dma_scratch_size=scratch,
                   num_swdge_queues=4)
    KM = FM // P
    NI = C * P   # gather indices per dst tile (includes the bias slot)
    NR = NP + 1  # gather tables have a bias row at NP

    d = {}
    d["xT1"] = nc.dram_tensor("xT1", [3, NP], BF16, kind="ExternalInput")
    d["hcrow"] = nc.dram_tensor("hcrow", [1, FM], BF16, kind="ExternalInput")
    d["W1v"] = nc.dram_tensor("W1v", [3, FM], BF16, kind="ExternalInput")
    for i in (2, 3, 4):
        d[f"W{i}"] = nc.dram_tensor(f"W{i}", [FM, FM], BF16, kind="ExternalInput")
    d["W5"] = nc.dram_tensor("W5", [FM, F5], BF16, kind="ExternalInput")
    d["W6"] = nc.dram_tensor("W6", [F5, FO], BF16, kind="ExternalInput")
    d["Brows"] = nc.dram_tensor("Brows", [4, FM], BF16, kind="ExternalInput")
    d["Brows56"] = nc.dram_tensor("Brows56", [2, P], BF16, kind="ExternalInput")
    d["b6rep"] = nc.dram_tensor("b6rep", [P, FO], F32, kind="ExternalInput")
    d["idx16"] = nc.dram_tensor("idx16", [P, T * SW], I16, kind="ExternalInput")
    d["slotb"] = nc.dram_tensor("slotb", [P, T * C], BF16, kind="ExternalInput")
    d["normb"] = nc.dram_tensor("normb", [P, T * C], BF16, kind="ExternalInput")
    d["dinv2"] = nc.dram_tensor("dinv2", [P, T], F32, kind="ExternalInput")
    out_d = nc.dram_tensor("out", [NP, FO], F32, kind="ExternalOutput")

    h512 = [nc.dram_tensor(f"h{i}", [NR, FM], BF16, kind="Internal")
            for i in (1, 2, 3, 4)]
    h5t = nc.dram_tensor("h5t", [NR, P], BF16, kind="Internal")
    x6t = nc.dram_tensor("x6t", [NR, P], BF16, kind="Internal")

    Ident = mybir.ActivationFunctionType.Identity
    Relu = mybir.ActivationFunctionType.Relu

    with tile.TileContext(nc) as tc:
        with ExitStack() as ctx:
            res = ctx.enter_context(tc.tile_pool(name="res", bufs=1))
            idx_sb = res.tile([P, T * SW], I16)
            slot_sb = res.tile([P, T * C], BF16)
            norm_sb = res.tile([P, T * C], BF16)
            dinv2_sb = res.tile([P, T], F32)
            b6r_sb = res.tile([P, FO], F32)
            hcrow_sb = res.tile([1, FM], BF16)
            brows_sb = res.tile([4, FM], BF16)
            brows56_sb = res.tile([2, P], BF16)
            for name, t_sb in [("idx16", idx_sb), ("slotb", slot_sb),
                               ("normb", norm_sb), ("dinv2", dinv2_sb),
                               ("b6rep", b6r_sb), ("hcrow", hcrow_sb),
                               ("Brows", brows_sb), ("Brows56", brows56_sb)]:
                nc.sync.dma_start(out=t_sb[:], in_=d[name][:, :])
            reg_ni = nc.gpsimd.to_reg(NI)
            qctr = [0]   # keeps queue_num in lock-step with DMASW lanes
            iota_i = res.tile([P, P], mybir.dt.int32)
            nc.gpsimd.iota(iota_i[:], pattern=[[1, P]], base=0, channel_multiplier=0)
            iota_b = res.tile([P, P], BF16)
            nc.vector.tensor_copy(out=iota_b[:], in_=iota_i[:])
            ident_b = res.tile([P, P], BF16)
            make_identity(nc, ident_b[:])
            hres = [res.tile([P, FM], BF16, name=f"hres_{t}") for t in range(T)]
            hres5 = [res.tile([P, P], BF16, name=f"hres5_{t}") for t in range(T)]
            for t in range(T):
                nc.vector.memset(hres5[t][:, F5:P], 0.0)

            def gather(sp, h_src, t, width, tag):
                g = sp.tile([P, C * width], BF16, tag=tag, name=f"{tag}_{t}")
                nc.gpsimd.dma_gather(
                    out_ap=g[:].rearrange("p (c f) -> p c f", c=C),
                    in_ap=h_src[:, :],
                    idxs_ap=idx_sb[:, t * SW:(t + 1) * SW],
                    num_idxs=NI, num_idxs_reg=reg_ni, elem_size=width,
                    queue_num=qctr[0] % 4)
                qctr[0] += 1
                return g

            def load_oh(sp, t, tag):
                # one-hot built on DVE: (slot == iota) * norm, then the
                # all-ones bias row at the fixed slot (e=127, c=C-1)
                oh = sp.tile([P, NI], BF16, tag=tag, name=f"{tag}_{t}")
                oh3 = oh[:].rearrange("p (c j) -> p c j", c=C)
                nc.vector.tensor_tensor(
                    out=oh3,
                    in0=slot_sb[:, t * C:(t + 1) * C]
                        .rearrange("p (c u) -> p c u", u=1).to_broadcast([P, C, P]),
                    in1=iota_b[:].rearrange("p (u j) -> p u j", u=1)
                        .to_broadcast([P, C, P]),
                    op=mybir.AluOpType.is_equal,
                )
                nc.vector.tensor_tensor(
                    out=oh3, in0=oh3,
                    in1=norm_sb[:, t * C:(t + 1) * C]
                        .rearrange("p (c u) -> p c u", u=1).to_broadcast([P, C, P]),
                    op=mybir.AluOpType.mult,
                )
                nc.vector.memset(oh[0:1, (C - 1) * P:C * P], 1.0)
                return oh

            # ---- layer 1 dense: h1 = (verts @ W1[:3] + img@W1[3:]) * dinv2 ----
            with tc.tile_pool(name="l1", bufs=1) as l1p, \
                 tc.tile_pool(name="l1ps", bufs=2, space="PSUM") as l1ps:
                nc.sync.dma_start(out=h512[0][NP:NP + 1, :], in_=brows_sb[0:1, :])
                xT1_sb = l1p.tile([3, NP], BF16)
                nc.sync.dma_start(out=xT1_sb[:], in_=d["xT1"][:, :])
                W1v_sb = l1p.tile([3, FM], BF16)
                nc.sync.dma_start(out=W1v_sb[:], in_=d["W1v"][:, :])
                ones1 = l1p.tile([1, P], BF16)
                nc.vector.memset(ones1[:], 1.0)
                for t in range(T):
                    pd1 = l1ps.tile([P, FM], F32, tag="pd1", name=f"pd1_{t}")
                    nc.tensor.matmul(out=pd1[:], lhsT=xT1_sb[:, t * P:(t + 1) * P],
                                     rhs=W1v_sb[:], start=True, stop=False)
                    nc.tensor.matmul(out=pd1[:], lhsT=ones1[:], rhs=hcrow_sb[:],
                                     start=False, stop=True)
                    nc.scalar.activation(out=hres[t][:], in_=pd1[:], func=Ident,
                                         scale=dinv2_sb[:, t:t + 1])
                    nc.sync.dma_start(out=h512[0][t * P:(t + 1) * P, :],
                                      in_=hres[t][:])

            # ---- merged phases: scatter(i) + dense(i+1), i = 1..4 ----
            import os as _os
            SBUFS = int(_os.environ.get("KBASS_BUFS", "7"))
            for i in (1, 2, 3, 4):
                relu = i in (2, 4)
                h_src = h512[i - 1]
                F_out = FM if i < 4 else F5
                W_d = d[f"W{i + 1}"]
                with tc.tile_pool(name=f"ph{i}", bufs=SBUFS) as sp, \
                     tc.tile_pool(name=f"ph{i}w", bufs=1) as wp, \
                     tc.tile_pool(name=f"ph{i}ps", bufs=3, space="PSUM") as pp, \
                     tc.tile_pool(name=f"ph{i}pt", bufs=2, space="PSUM") as pt, \
                     tc.tile_pool(name=f"ph{i}pd", bufs=2, space="PSUM") as pd:
                    # bias row of the NEXT phase's gather table
                    if i < 4:
                        nc.sync.dma_start(out=h512[i][NP:NP + 1, :],
                                          in_=brows_sb[i:i + 1, :])
                    else:
                        nc.sync.dma_start(out=h5t[NP:NP + 1, :],
                                          in_=brows56_sb[0:1, :])
                    W_sb = [wp.tile([P, F_out], BF16, tag=f"w{k}", name=f"w{i}_{k}")
                            for k in range(KM)]
                    for k in range(KM):
                        nc.sync.dma_start(out=W_sb[k][:], in_=W_d[k * P:(k + 1) * P, :])
                    for t in range(T):
                        g = gather(sp, h_src, t, FM, f"g{i}")
                        oh = load_oh(sp, t, f"oh{i}")
                        pa = pp.tile([P, FM], F32, tag="pa", name=f"pa{i}_{t}")
                        for c in range(C):
                            nc.tensor.matmul(
                                out=pa[:], lhsT=oh[:, c * P:(c + 1) * P],
                                rhs=g[:, c * FM:(c + 1) * FM],
                                start=(c == 0), stop=(c == C - 1))
                        # self-loop (h rows pre-scaled by dinv^2) fused into
                        # the PSUM->SBUF drain on DVE; bias came via the
                        # all-ones one-hot slot
                        node = sp.tile([P, FM], BF16, tag="node", name=f"nd{i}_{t}")
                        nc.vector.tensor_add(out=node[:], in0=pa[:],
                                             in1=hres[t][:])
                        ptr = pt.tile([P, FM], F32, tag="ptr", name=f"pt{i}_{t}")
                        for fo in range(KM):
                            nc.tensor.matmul(
                                out=ptr[:, fo * P:(fo + 1) * P],
                                lhsT=node[:, fo * P:(fo + 1) * P],
                                rhs=ident_b[:],
                                start=True, stop=True)
                        stage = sp.tile([P, FM], BF16, tag="stage", name=f"st{i}_{t}")
                        nc.scalar.activation(out=stage[:], in_=ptr[:],
                                             func=Relu if relu else Ident)
                        pdn = pd.tile([P, F_out], F32, tag="pdn", name=f"pd{i}_{t}")
                        for k in range(KM):
                            nc.tensor.matmul(out=pdn[:],
                                             lhsT=stage[:, k * P:(k + 1) * P],
                                             rhs=W_sb[k][:], start=(k == 0),
                                             stop=(k == KM - 1))
                        if i < 4:
                            nc.scalar.activation(out=hres[t][:], in_=pdn[:],
                                                 func=Ident,
                                                 scale=dinv2_sb[:, t:t + 1])
                            nc.sync.dma_start(out=h512[i][t * P:(t + 1) * P, :],
                                              in_=hres[t][:])
                        else:
                            nc.scalar.activation(out=hres5[t][:, 0:F5],
                                                 in_=pdn[:], func=Ident,
                                                 scale=dinv2_sb[:, t:t + 1])
                            nc.sync.dma_start(out=h5t[t * P:(t + 1) * P, :],
                                              in_=hres5[t][:])

            # ---- phase 5: x6 = (A_hat h5 + b5) * dinv2, 64-wide ----
            with tc.tile_pool(name="s5", bufs=SBUFS) as sp5, \
                 tc.tile_pool(name="s5ps", bufs=2, space="PSUM") as pp5, \
                 tc.tile_pool(name="s5pt", bufs=2, space="PSUM") as pt5:
                nc.sync.dma_start(out=x6t[NP:NP + 1, :], in_=brows56_sb[1:2, :])
                for t in range(T):
                    g5 = gather(sp5, h5t, t, P, "g5")
                    oh5 = load_oh(sp5, t, "oh5")
                    pg5 = pp5.tile([F5, P], F32, tag="pg5", name=f"pg5_{t}")
                    nc.tensor.matmul(out=pg5[:], lhsT=hres5[t][:, 0:F5],
                                     rhs=ident_b[:],
                                     start=True, stop=False,
                                     skip_group_check=True)
                    for c in range(C):
                        nc.tensor.matmul(out=pg5[:],
                                         lhsT=g5[:, c * P:c * P + F5],
                                         rhs=oh5[:, c * P:(c + 1) * P],
                                         start=False, stop=(c == C - 1),
                                         skip_group_check=True)
                    st6 = sp5.tile([F5, P], BF16, tag="st6", name=f"st6_{t}")
                    nc.scalar.activation(out=st6[:], in_=pg5[:], func=Ident)
                    pt6 = pt5.tile([P, F5], F32, tag="pt6", name=f"pt6_{t}")
                    nc.tensor.matmul(out=pt6[:], lhsT=st6[:],
                                     rhs=ident_b[0:F5, 0:F5],
                                     start=True, stop=True)
                    nc.scalar.activation(out=hres5[t][:, 0:F5], in_=pt6[:],
                                         func=Ident,
                                         scale=dinv2_sb[:, t:t + 1])
                    nc.sync.dma_start(out=x6t[t * P:(t + 1) * P, :],
                                      in_=hres5[t][:])

            # ---- phase 6: out = (A_hat x6) @ W6 + b6 ----
            with tc.tile_pool(name="s6", bufs=SBUFS) as sp6, \
                 tc.tile_pool(name="s6w", bufs=1) as wp6, \
                 tc.tile_pool(name="s6ps", bufs=2, space="PSUM") as pp6, \
                 tc.tile_pool(name="s6pd", bufs=2, space="PSUM") as po6:
                W6_sb = wp6.tile([F5, FO], BF16)
                nc.sync.dma_start(out=W6_sb[:], in_=d["W6"][:, :])
                for t in range(T):
                    g6 = gather(sp6, x6t, t, P, "g6")
                    oh6 = load_oh(sp6, t, "oh6")
                    pg6 = pp6.tile([F5, P], F32, tag="pg6", name=f"pg6_{t}")
                    nc.tensor.matmul(out=pg6[:], lhsT=hres5[t][:, 0:F5],
                                     rhs=ident_b[:],
                                     start=True, stop=False,
                                     skip_group_check=True)
                    for c in range(C):
                        nc.tensor.matmul(out=pg6[:],
                                         lhsT=g6[:, c * P:c * P + F5],
                                         rhs=oh6[:, c * P:(c + 1) * P],
                                         start=False, stop=(c == C - 1),
                                         skip_group_check=True)
                    st7 = sp6.tile([F5, P], BF16, tag="st7", name=f"st7_{t}")
                    nc.scalar.activation(out=st7[:], in_=pg6[:], func=Ident)
                    pout = po6.tile([P, FO], F32, tag="pout", name=f"po_{t}")
                    nc.tensor.matmul(out=pout[:], lhsT=st7[:], rhs=W6_sb[:],
                                     start=True, stop=True)
                    os_ = sp6.tile([P, FO], F32, tag="os", name=f"o_{t}")
                    nc.vector.tensor_add(out=os_[:], in0=pout[:], in1=b6r_sb[:])
                    nc.sync.dma_start(out=out_d[t * P:(t + 1) * P, :], in_=os_[:])

    nc.compile()
    return nc


def _prepare(batch_vertices, img_features, edge_indices,
             W1, b1, W2, b2, W3, b3, W4, b4, W5, b5, W6, b6):
    B, N, _ = batch_vertices.shape
    FM = W1.shape[1]
    F5 = W5.shape[1]
    FO = W6.shape[1]

    ei = np.asarray(edge_indices).astype(np.int64)
    g = _pack_graph(ei[0], ei[1], N)
    NP, T, C, SW, perm = g["NP"], g["T"], g["C"], g["SW"], g["perm"]

    hc = img_features.astype(np.float32) @ W1[3:].astype(np.float32)

    valid = perm >= 0
    vperm = np.zeros((B, NP, 3), np.float32)
    vperm[:, valid, :] = batch_vertices[:, perm[valid], :]

    b5row = np.zeros((1, P), np.float32)
    b5row[0, :F5] = b5
    common = {
        "W1v": np.ascontiguousarray(W1[:3]).astype(BF),
        "W2": np.ascontiguousarray(W2).astype(BF),
        "W3": np.ascontiguousarray(W3).astype(BF),
        "W4": np.ascontiguousarray(W4).astype(BF),
        "W5": np.ascontiguousarray(W5).astype(BF),
        "W6": np.ascontiguousarray(W6).astype(BF),
        "Brows": np.stack([b1, b2, b3, b4]).astype(BF),
        "Brows56": np.concatenate([b5row, np.zeros((1, P), np.float32)]).astype(BF),
        "b6rep": np.tile(b6.astype(np.float32), (P, 1)),
        "idx16": g["idx16"], "slotb": g["slot"], "normb": g["norm"],
        "dinv2": g["dinv2"],
    }
    in_maps = []
    for b in range(B):
        m = dict(common)
        m["xT1"] = np.ascontiguousarray(vperm[b].T).astype(BF)
        m["hcrow"] = hc[b].reshape(1, FM).astype(BF)
        in_maps.append(m)
    meta = dict(NP=NP, T=T, C=C, SW=SW, perm=perm, valid=valid, B=B, N=N,
                FM=FM, F5=F5, FO=FO)
    return in_maps, meta


_BUILD_CACHE = {}


def run(inputs, trace=False):
    in_maps, meta = _prepare(**inputs)
    key = (meta["NP"], meta["C"], meta["FM"], meta["F5"], meta["FO"])
    if key not in _BUILD_CACHE:
        t0 = time.time()
        _BUILD_CACHE[key] = _build_nc(meta["NP"], meta["T"], meta["C"],
                                      meta["SW"], meta["FM"], meta["F5"],
                                      meta["FO"])
        print(f"[kernel] built bass program in {time.time()-t0:.1f}s", file=sys.stderr)
    nc = _BUILD_CACHE[key]
    B = meta["B"]
    res = run_bass_kernel_spmd(nc, in_maps, core_ids=list(range(B)), trace=trace)
    perm, valid, N = meta["perm"], meta["valid"], meta["N"]
    out = np.empty((B, N, meta["FO"]), np.float32)
    for b in range(B):
        dev = res.results[b]["out"]
        out[b, perm[valid], :] = dev[valid, :]
    return out, res


def kernel(**inputs) -> np.ndarray:
    out, _ = run(inputs)
    return out

